# revision 30
# baseline (speedup 1.0000x reference)
"""MultiHeadAttention kernel for Trainium2, 8-core head-parallel.

Problem: S=2048, B=2, D=1024, 16 heads of d=64 (batch_first=False).
Sharding: tensor-parallel over heads - each of the 8 cores computes 2 heads.

Per-core dataflow (v2 - engine-balanced):
  q^T, k^T  [128, T] = W_slice @ x^T   (PE, bf16; bias folded into the
            psum->SBUF evacuation on ScalarE/VectorE)
  v'        [tok, 64] per head via DMA-xbar transpose of v^T (no PE/DVE)
  scores    [j, i] per head; the two heads' K=64 matmuls are emitted
            adjacently at row groups 0/64 so they run concurrently.
  exp       h0 on ScalarE (table exp); h1 on VectorE via a Schraudolph
            bit-trick: bits16 = R*(128*log2e/8) + (16256.5-4.5) computed
            by one tensor_scalar into an int16 view of the bf16 tile.
  pv        h0 -> psum E rows 0:64 (col groups 0-1), h1 -> E rows 64:128
            (col groups 2-3, tile_position=(0,64)) - concurrent.
  den       ones[128,1] matmuls into psum F rows 0 / 32 (col groups 0/1).
  out       E,F evacuated raw (numerators + denominators); the host
            divides during the gather (no reciprocal on device).
"""

import sys

if "/opt/trn_rl_repo" not in sys.path:
    sys.path.insert(0, "/opt/trn_rl_repo")

import numpy as np
import ml_dtypes

import concourse.bass as bass
import concourse.mybir as mybir
import concourse.tile as tile
from concourse import bacc

BF16 = mybir.dt.bfloat16
FP32 = mybir.dt.float32
I16 = mybir.dt.int16
NP_BF16 = ml_dtypes.bfloat16

D = 1024
NHEAD = 16
DH = 64
NCORES = 8
HPC = NHEAD // NCORES        # heads per core = 2
DC = HPC * DH                # per-core projection dims = 128
KT = D // 128                # contraction tiles = 8
SCALE = 1.0 / float(np.sqrt(DH))
# Schraudolph exp in bf16 bits: bits = R*SCH_A + SCH_B, bitcast int16->bf16
SCH_A = 128.0 * float(np.log2(np.e)) * SCALE
SCH_B = 128.0 * 127.0 + 0.5 - 4.5


def build_program(S: int, B: int):
    assert S % 512 == 0
    T = S * B
    JT = S // 128                  # j tiles per batch (16)
    IC = 1024                      # i-chunk width (psum scores tile)
    NIC = S // IC                  # i-chunks per batch (2)
    TB = 512                       # tokens per projection unit
    TPB = S // TB                  # proj units per (proj, batch) (4)
    NTILE = T // TB

    nc = bacc.Bacc(
        "TRN2", target_bir_lowering=False, debug=False, num_devices=NCORES
    )
    xq = nc.dram_tensor("xq", [NTILE, 128, KT, TB], BF16, kind="ExternalInput")
    xk = nc.dram_tensor("xk", [NTILE, 128, KT, TB], BF16, kind="ExternalInput")
    xv = nc.dram_tensor("xv", [NTILE, 128, KT, TB], BF16, kind="ExternalInput")
    # pre-tiled on host: [128, KT, DC] so the load is one dense DMA
    wq = nc.dram_tensor("wq", [128, KT, DC], BF16, kind="ExternalInput")
    wk = nc.dram_tensor("wk", [128, KT, DC], BF16, kind="ExternalInput")
    wv = nc.dram_tensor("wv", [128, KT, DC], BF16, kind="ExternalInput")
    bqkv = nc.dram_tensor("bqkv", [DC, 3], FP32, kind="ExternalInput")
    # rows 0:64 pv h0, 64:128 pv h1, 128 den h0, 129 den h1
    out = nc.dram_tensor("out", [130, T], FP32, kind="ExternalOutput")

    with tile.TileContext(nc) as tc:
        with (
            tc.tile_pool(name="const", bufs=1) as constp,
            tc.tile_pool(name="xin", bufs=1) as xinp,
            tc.tile_pool(name="qkv", bufs=1) as qkvp,
            tc.tile_pool(name="attn", bufs=1) as attnp,
            tc.tile_pool(name="vstg", bufs=2) as vstgp,
            tc.tile_pool(name="outp", bufs=2) as outp,
            tc.tile_pool(name="sc0", bufs=1, space="PSUM") as sc0p,
            tc.tile_pool(name="sc1", bufs=1, space="PSUM") as sc1p,
            tc.tile_pool(name="pvp", bufs=1, space="PSUM") as pvp,
            tc.tile_pool(name="dnp", bufs=1, space="PSUM") as dnp,
            tc.tile_pool(name="prj", bufs=1, space="PSUM") as prjp,
        ):
            wq_t = constp.tile([128, KT, DC], BF16, tag="wq")
            wk_t = constp.tile([128, KT, DC], BF16, tag="wk")
            wv_t = constp.tile([128, KT, DC], BF16, tag="wv")
            nc.sync.dma_start(out=wk_t[:], in_=wk[:, :, :])
            nc.gpsimd.dma_start(out=wq_t[:], in_=wq[:, :, :])
            nc.scalar.dma_start(out=wv_t[:], in_=wv[:, :, :])
            bqkv_t = constp.tile([DC, 3], FP32, tag="bqkv")
            ones_t = constp.tile([128, 1], BF16, tag="ones")
            nc.vector.memset(ones_t[:], 1.0)

            q_b, k_b, v_all = [], [], []
            for b in range(B):
                q_b.append(qkvp.tile([128, S], BF16, tag=f"q{b}", name=f"q{b}"))
                k_b.append(qkvp.tile([128, S], BF16, tag=f"k{b}", name=f"k{b}"))
                # one token-major v tile per j-tile: [tok, 128 vdims] with
                # h0 dims in cols 0:64, h1 in 64:128 (precise PV deps)
                v_all.append([
                    qkvp.tile([128, 128], BF16, tag=f"v{b}j{j}", name=f"v{b}j{j}")
                    for j in range(JT)
                ])
            tr_alt = [0]

            late_consts = [False]
            qk_alt = [0]

            def _proj_qk(b, tb, xsrc, xtag, w_t, bcol, dst, warmup=False):
                s0 = tb * TB
                x_t = xinp.tile([128, KT, TB], BF16, tag=xtag, name=xtag, bufs=2)
                # k on sync, q on gpsimd, v on scalar: the lead-in
                # projection chains stream their inputs concurrently
                eng = {"xk": nc.sync, "xq": nc.gpsimd, "xv": nc.scalar}[xtag]
                eng.dma_start(out=x_t[:], in_=xsrc[b * TPB + tb, :, :, :])
                if warmup and not late_consts[0]:
                    late_consts[0] = True
                    nc.sync.dma_start(out=bqkv_t[:], in_=bqkv[:, :])
                    # weight-only HAM warm-up: fires as soon as wk lands,
                    # well before the first 1MB x tile arrives
                    warm = prjp.tile([128, TB], FP32, tag="prj", name="warm")
                    for _ in range(32):
                        nc.tensor.matmul(
                            warm[:, :128], w_t[:, 0, :], w_t[:, 0, 0:128],
                            start=True, stop=True,
                        )
                ps_x = prjp.tile([128, TB], FP32, tag="prj", name="ps_x")
                for kt in range(KT):
                    nc.tensor.matmul(
                        ps_x[:, :], w_t[:, kt, :], x_t[:, kt, :],
                        start=(kt == 0), stop=(kt == KT - 1),
                    )
                # evacuate + bias on alternating engines (both are exp-loaded;
                # spread the cost)
                if qk_alt[0] % 4 != 3:
                    nc.scalar.activation(
                        out=dst[:, s0 : s0 + TB], in_=ps_x[:, :],
                        func=mybir.ActivationFunctionType.Identity,
                        bias=bqkv_t[:, bcol : bcol + 1],
                    )
                else:
                    nc.vector.tensor_scalar(
                        dst[:, s0 : s0 + TB], ps_x[:, :],
                        bqkv_t[:, bcol : bcol + 1], None,
                        mybir.AluOpType.add,
                    )
                qk_alt[0] += 1

            def emit_proj_q(b, tb, warmup=False):
                _proj_qk(b, tb, xq, "xq", wq_t, 0, q_b[b], warmup=warmup)

            def emit_proj_k(b, tb, warmup=False):
                _proj_qk(b, tb, xk, "xk", wk_t, 1, k_b[b], warmup=warmup)

            def emit_proj_v(b, tb):
                # v^T [128, TB] in psum -> +bias -> vT staging sbuf ->
                # 8 DMA-xbar transposes into the per-head token-major tiles.
                x_t = xinp.tile([128, KT, TB], BF16, tag="xv", name="xv_t", bufs=2)
                nc.gpsimd.dma_start(out=x_t[:], in_=xv[b * TPB + tb, :, :, :])
                ps_v = prjp.tile([128, TB], FP32, tag="prj", name="ps_v")
                for kt in range(KT):
                    nc.tensor.matmul(
                        ps_v[:, :], wv_t[:, kt, :], x_t[:, kt, :],
                        start=(kt == 0), stop=(kt == KT - 1),
                    )
                vT = vstgp.tile([128, TB], BF16, tag="vT", name="vT")
                nc.vector.tensor_scalar(
                    vT[:, :], ps_v[:, :], bqkv_t[:, 2:3], None,
                    mybir.AluOpType.add,
                )
                for sub in range(TB // 128):
                    jt = tb * (TB // 128) + sub
                    nc.scalar.dma_start_transpose(
                        out=v_all[b][jt][:, :],
                        in_=vT[:, sub * 128 : (sub + 1) * 128],
                    )

            def emit_attention(b, inject=None):
                for ic in range(NIC):
                    at0 = attnp.tile([128, JT, IC], BF16, tag="at0", name="at0")
                    at1 = attnp.tile([128, JT, IC], BF16, tag="at1", name="at1")
                    pv_ps = pvp.tile([128, 2, 512], FP32, tag="pv", name="pv")
                    # one-bank den tile: 4 accumulation chains at partition
                    # offsets 0/32/64/96 = (h0,ih0),(h1,ih0),(h0,ih1),(h1,ih1)
                    dn_ps = dnp.tile([97, 512], FP32, tag="dn", name="dn")
                    pending = []

                    def make_pv(jt, at0=at0, at1=at1, pv_ps=pv_ps, dn_ps=dn_ps, b=b):
                        def go():
                            # pv pairs (col groups 0-1 vs 2-3) adjacent, then
                            # all four den chains (col groups 0/1/2/3) adjacent
                            for ih in range(2):
                                rh0 = at0[:, jt, ih * 512 : (ih + 1) * 512]
                                rh1 = at1[:, jt, ih * 512 : (ih + 1) * 512]
                                nc.tensor.matmul(
                                    pv_ps[0:DH, ih, :], v_all[b][jt][:, 0:DH], rh0,
                                    start=(jt == 0), stop=(jt == JT - 1),
                                    tile_position=(0, 0),
                                )
                                nc.tensor.matmul(
                                    pv_ps[DH:128, ih, :], v_all[b][jt][:, DH:128], rh1,
                                    start=(jt == 0), stop=(jt == JT - 1),
                                    tile_position=(0, 64),
                                )
                            for ih in range(2):
                                rh0 = at0[:, jt, ih * 512 : (ih + 1) * 512]
                                rh1 = at1[:, jt, ih * 512 : (ih + 1) * 512]
                                p0 = 64 * ih
                                nc.tensor.matmul(
                                    dn_ps[p0 : p0 + 1, :], ones_t[:, 0:1], rh0,
                                    start=(jt == 0), stop=(jt == JT - 1),
                                    tile_position=(0, p0),
                                )
                                nc.tensor.matmul(
                                    dn_ps[p0 + 32 : p0 + 33, :], ones_t[:, 0:1], rh1,
                                    start=(jt == 0), stop=(jt == JT - 1),
                                    tile_position=(0, p0 + 32),
                                )
                        return go

                    for jt in range(JT):
                        if inject is not None:
                            inject(ic, jt)
                        s0 = sc0p.tile([128, IC], FP32, tag="s0", name="s0")
                        s1 = sc1p.tile([128, IC], FP32, tag="s1", name="s1")
                        # adjacent emission -> the two heads' K=64 matmuls
                        # run concurrently in row groups 0-63 / 64-127
                        for n in range(IC // 512):
                            i0 = ic * IC + n * 512
                            nc.tensor.matmul(
                                s0[:, n * 512 : (n + 1) * 512],
                                k_b[b][0:DH, jt * 128 : (jt + 1) * 128],
                                q_b[b][0:DH, i0 : i0 + 512],
                                start=True, stop=True,
                            )
                            nc.tensor.matmul(
                                s1[:, n * 512 : (n + 1) * 512],
                                k_b[b][DH:128, jt * 128 : (jt + 1) * 128],
                                q_b[b][DH:128, i0 : i0 + 512],
                                start=True, stop=True,
                            )
                        # h0: table exp on ScalarE; h1: Schraudolph on VectorE
                        nc.scalar.activation(
                            out=at0[:, jt, :], in_=s0[:, :],
                            func=mybir.ActivationFunctionType.Exp,
                            scale=SCALE,
                        )
                        nc.vector.tensor_scalar(
                            at1[:, jt, :].bitcast(I16), s1[:, :],
                            SCH_A, SCH_B,
                            mybir.AluOpType.mult, mybir.AluOpType.add,
                        )
                        # pv/den of jt-1 AFTER this jt's scores+exp dispatch:
                        # their deps are ready, and the exp round-trip (which
                        # sets the period) isn't delayed behind them
                        if pending:
                            pending.pop()()
                        pending.append(make_pv(jt))
                    while pending:
                        pending.pop()()
                    # evacuate numerators (ScalarE) + denominators (VectorE)
                    pvsb = outp.tile([128, 2, 512], FP32, tag="pvsb", name="pvsb")
                    nc.scalar.copy(pvsb[:, :, :], pv_ps[:, :, :])
                    dnsb = outp.tile([97, 512], FP32, tag="dnsb", name="dnsb")
                    nc.vector.tensor_copy(dnsb[:, :], dn_ps[:, :])
                    c0 = b * S + ic * IC
                    nc.gpsimd.dma_start(
                        out=out[0:128, c0 : c0 + IC],
                        in_=pvsb[:, :, :].rearrange("p a b -> p (a b)"),
                    )
                    for ih in range(2):
                        p0 = 64 * ih
                        nc.sync.dma_start(
                            out=out[128:129, c0 + ih * 512 : c0 + (ih + 1) * 512],
                            in_=dnsb[p0 : p0 + 1, :],
                        )
                        nc.sync.dma_start(
                            out=out[129:130, c0 + ih * 512 : c0 + (ih + 1) * 512],
                            in_=dnsb[p0 + 32 : p0 + 33, :],
                        )

            # ---- schedule ----
            # Pre-attention: k/q for the first i-chunk and j-tiles 0:8,
            # first v unit; the rest injected with >=4 points of slack
            # before their first consumer (the PE pulls LDWEIGHTS ahead
            # of in-flight matmuls, so tight evac->read gaps are unsafe).
            emit_proj_k(0, 0, warmup=True)
            emit_proj_q(0, 0)
            emit_proj_k(0, 1)
            emit_proj_q(0, 1)
            emit_proj_v(0, 0)
            emit_proj_v(0, 1)

            units0 = [
                (0, lambda: emit_proj_v(0, 2)),
                (2, lambda: emit_proj_k(0, 2)),
                (4, lambda: emit_proj_v(0, 3)),
                (6, lambda: emit_proj_k(0, 3)),
                (8, lambda: emit_proj_q(0, 2)),
                (12, lambda: emit_proj_q(0, 3)),
            ]
            units1 = []
            if B > 1:
                pts = [13, 15, 17, 19, 21, 23, 25, 27, 29, 31]
                seq = [
                    lambda: emit_proj_k(1, 0), lambda: emit_proj_k(1, 1),
                    lambda: emit_proj_k(1, 2), lambda: emit_proj_k(1, 3),
                    lambda: emit_proj_v(1, 0), lambda: emit_proj_v(1, 1),
                    lambda: emit_proj_q(1, 0), lambda: emit_proj_q(1, 1),
                    lambda: emit_proj_v(1, 2), lambda: emit_proj_v(1, 3),
                ]
                units0 += list(zip(pts, seq))
                units1 = [
                    (2, lambda: emit_proj_q(1, 2)),
                    (6, lambda: emit_proj_q(1, 3)),
                ]

            def make_inject(units):
                units = sorted(units, key=lambda u: u[0])
                ui = [0]

                def inject(ic, jt):
                    point = ic * JT + jt
                    while ui[0] < len(units) and units[ui[0]][0] <= point:
                        units[ui[0]][1]()
                        ui[0] += 1

                def flush():
                    while ui[0] < len(units):
                        units[ui[0]][1]()
                        ui[0] += 1

                return inject, flush

            inj0, flush0 = make_inject(units0)
            emit_attention(0, inject=inj0)
            flush0()
            if B > 1:
                inj1, flush1 = make_inject(units1)
                emit_attention(1, inject=inj1)
                flush1()

    nc.finalize()
    return nc


_PROGRAM_CACHE = {}


def _get_program(S, B):
    key = (S, B)
    if key not in _PROGRAM_CACHE:
        _PROGRAM_CACHE[key] = build_program(S, B)
    return _PROGRAM_CACHE[key]


def make_in_maps(query, key, value, Wq, bq, Wk, bk, Wv, bv):
    S, B, D_ = query.shape
    assert D_ == D
    T = S * B
    TB = 512
    NTILE = T // TB

    def xt(a):
        aT = np.asarray(a, np.float32).transpose(2, 1, 0).reshape(D_, T)
        a4 = aT.reshape(KT, 128, NTILE, TB).transpose(2, 1, 0, 3)
        return np.ascontiguousarray(a4).astype(NP_BF16)

    xqh, xkh, xvh = xt(query), xt(key), xt(value)

    def wt(W, rows):
        # [D, DC] col-slice -> [128, KT, DC] (partition-major contraction)
        wT = np.asarray(W)[rows, :].T.reshape(KT, 128, DC).transpose(1, 0, 2)
        return np.ascontiguousarray(wT).astype(NP_BF16)

    in_maps = []
    for c in range(NCORES):
        rows = slice(c * DC, (c + 1) * DC)
        in_maps.append(
            {
                "xq": xqh, "xk": xkh, "xv": xvh,
                "wq": wt(Wq, rows),
                "wk": wt(Wk, rows),
                "wv": wt(Wv, rows),
                "bqkv": np.ascontiguousarray(
                    np.stack(
                        [np.asarray(bq)[rows], np.asarray(bk)[rows], np.asarray(bv)[rows]],
                        axis=1,
                    )
                ).astype(np.float32),
            }
        )
    return in_maps


def gather_output(results, S, B):
    full = np.empty((S, B, D), np.float32)
    for c in range(NCORES):
        o = np.asarray(results[c]["out"], np.float32)  # [130, B*S]
        num = o[0:128]                                 # [128, T]
        den = np.empty((128, S * B), np.float32)
        den[0:DH] = o[128:129]
        den[DH:128] = o[129:130]
        res = (num / den).reshape(128, B, S).transpose(2, 1, 0)
        full[:, :, c * DC : (c + 1) * DC] = res
    return full


def kernel(query, key, value, Wq, bq, Wk, bk, Wv, bv):
    from concourse.bass_utils import run_bass_kernel_spmd

    S, B, _ = query.shape
    nc = _get_program(S, B)
    in_maps = make_in_maps(query, key, value, Wq, bq, Wk, bk, Wv, bv)
    res = run_bass_kernel_spmd(nc, in_maps, list(range(NCORES)))
    return gather_output(res.results, S, B)


# revision 40
# speedup vs baseline: 1.2004x; 1.2004x over previous
"""MultiHeadAttention kernel for Trainium2, 8-core head-parallel.

Problem: S=2048, B=2, D=1024, 16 heads of d=64 (batch_first=False).
Sharding: tensor-parallel over heads - each of the 8 cores computes 2 heads.

Per-core dataflow (v2 - engine-balanced):
  q^T, k^T  [128, T] = W_slice @ x^T   (PE, bf16; bias folded into the
            psum->SBUF evacuation on ScalarE/VectorE)
  v'        [tok, 64] per head via DMA-xbar transpose of v^T (no PE/DVE)
  scores    [j, i] per head; the two heads' K=64 matmuls are emitted
            adjacently at row groups 0/64 so they run concurrently.
  exp       h0 on ScalarE (table exp); h1 on VectorE via a Schraudolph
            bit-trick: bits16 = R*(128*log2e/8) + (16256.5-4.5) computed
            by one tensor_scalar into an int16 view of the bf16 tile.
  pv        h0 -> psum E rows 0:64 (col groups 0-1), h1 -> E rows 64:128
            (col groups 2-3, tile_position=(0,64)) - concurrent.
  den       ones[128,1] matmuls into psum F rows 0 / 32 (col groups 0/1).
  out       E,F evacuated raw (numerators + denominators); the host
            divides during the gather (no reciprocal on device).
"""

import sys

if "/opt/trn_rl_repo" not in sys.path:
    sys.path.insert(0, "/opt/trn_rl_repo")

import numpy as np
import ml_dtypes

import concourse.bass as bass
import concourse.mybir as mybir
import concourse.tile as tile
from concourse import bacc

BF16 = mybir.dt.bfloat16
FP32 = mybir.dt.float32
I16 = mybir.dt.int16
NP_BF16 = ml_dtypes.bfloat16

D = 1024
NHEAD = 16
DH = 64
NCORES = 8
HPC = NHEAD // NCORES        # heads per core = 2
DC = HPC * DH                # per-core projection dims = 128
KT = D // 128                # contraction tiles = 8
SCALE = 1.0 / float(np.sqrt(DH))
# Schraudolph exp in bf16 bits: bits = R*SCH_A + SCH_B, bitcast int16->bf16
SCH_A = 128.0 * float(np.log2(np.e)) * SCALE
SCH_B = 128.0 * 127.0 + 0.5 - 4.5


def build_program(S: int, B: int):
    assert S % 512 == 0
    T = S * B
    JT = S // 128                  # j tiles per batch (16)
    IC = 1024                      # i-chunk width (psum scores tile)
    NIC = S // IC                  # i-chunks per batch (2)
    TB = 512                       # tokens per projection unit
    TPB = S // TB                  # proj units per (proj, batch) (4)
    NTILE = T // TB

    nc = bacc.Bacc(
        "TRN2", target_bir_lowering=False, debug=False, num_devices=NCORES
    )
    xq = nc.dram_tensor("xq", [NTILE, 128, KT, TB], BF16, kind="ExternalInput")
    xk = nc.dram_tensor("xk", [NTILE, 128, KT, TB], BF16, kind="ExternalInput")
    xv = nc.dram_tensor("xv", [NTILE, 128, KT, TB], BF16, kind="ExternalInput")
    # pre-tiled on host: [128, KT, DC] so the load is one dense DMA
    wq = nc.dram_tensor("wq", [128, KT, DC], BF16, kind="ExternalInput")
    wk = nc.dram_tensor("wk", [128, KT, DC], BF16, kind="ExternalInput")
    wv = nc.dram_tensor("wv", [128, KT, DC], BF16, kind="ExternalInput")
    bqkv = nc.dram_tensor("bqkv", [DC, 3], FP32, kind="ExternalInput")
    # rows 0:64 pv h0, 64:128 pv h1, 128 den h0, 129 den h1
    out = nc.dram_tensor("out", [130, T], FP32, kind="ExternalOutput")

    with tile.TileContext(nc) as tc:
        with (
            tc.tile_pool(name="const", bufs=1) as constp,
            tc.tile_pool(name="xin", bufs=1) as xinp,
            tc.tile_pool(name="qkv", bufs=1) as qkvp,
            tc.tile_pool(name="attn", bufs=1) as attnp,
            tc.tile_pool(name="vstg", bufs=2) as vstgp,
            tc.tile_pool(name="outp", bufs=2) as outp,
            tc.tile_pool(name="sc0", bufs=1, space="PSUM") as sc0p,
            tc.tile_pool(name="sc1", bufs=1, space="PSUM") as sc1p,
            tc.tile_pool(name="pvp", bufs=1, space="PSUM") as pvp,
            tc.tile_pool(name="dnp", bufs=1, space="PSUM") as dnp,
            tc.tile_pool(name="prj", bufs=1, space="PSUM") as prjp,
        ):
            wq_t = constp.tile([128, KT, DC], BF16, tag="wq")
            wk_t = constp.tile([128, KT, DC], BF16, tag="wk")
            wv_t = constp.tile([128, KT, DC], BF16, tag="wv")
            nc.sync.dma_start(out=wk_t[:], in_=wk[:, :, :])
            nc.sync.dma_start(out=wq_t[:], in_=wq[:, :, :])
            nc.sync.dma_start(out=wv_t[:], in_=wv[:, :, :])
            bqkv_t = constp.tile([DC, 3], FP32, tag="bqkv")
            ones_t = constp.tile([128, 1], BF16, tag="ones")
            nc.vector.memset(ones_t[:], 1.0)

            q_b, k_b, v_all = [], [], []
            for b in range(B):
                q_b.append(qkvp.tile([128, S], BF16, tag=f"q{b}", name=f"q{b}"))
                k_b.append(qkvp.tile([128, S], BF16, tag=f"k{b}", name=f"k{b}"))
                # one token-major v tile per j-tile: [tok, 128 vdims] with
                # h0 dims in cols 0:64, h1 in 64:128 (precise PV deps)
                v_all.append([
                    qkvp.tile([128, 128], BF16, tag=f"v{b}j{j}", name=f"v{b}j{j}")
                    for j in range(JT)
                ])
            tr_alt = [0]

            late_consts = [False]
            qk_alt = [0]

            def _proj_qk(b, tb, xsrc, xtag, w_t, bcol, dst, warmup=False):
                s0 = tb * TB
                x_t = xinp.tile([128, KT, TB], BF16, tag=xtag, name=xtag, bufs=2)
                # k on sync, q/v on gpsimd: the lead-in projection
                # chains stream their inputs concurrently
                eng = nc.sync if xtag == "xk" else nc.gpsimd
                eng.dma_start(out=x_t[:], in_=xsrc[b * TPB + tb, :, :, :])
                if warmup and not late_consts[0]:
                    late_consts[0] = True
                    nc.sync.dma_start(out=bqkv_t[:], in_=bqkv[:, :])
                    warm = prjp.tile([128, TB], FP32, tag="prj", name="warm")
                    for _ in range(32):
                        nc.tensor.matmul(
                            warm[:, :128], w_t[:, 0, :], x_t[:, 0, 0:128],
                            start=True, stop=True,
                        )
                ps_x = prjp.tile([128, TB], FP32, tag="prj", name="ps_x")
                for kt in range(KT):
                    nc.tensor.matmul(
                        ps_x[:, :], w_t[:, kt, :], x_t[:, kt, :],
                        start=(kt == 0), stop=(kt == KT - 1),
                    )
                # evacuate + bias on alternating engines (both are exp-loaded;
                # spread the cost)
                if qk_alt[0] % 4 != 3:
                    nc.scalar.activation(
                        out=dst[:, s0 : s0 + TB], in_=ps_x[:, :],
                        func=mybir.ActivationFunctionType.Identity,
                        bias=bqkv_t[:, bcol : bcol + 1],
                    )
                else:
                    nc.vector.tensor_scalar(
                        dst[:, s0 : s0 + TB], ps_x[:, :],
                        bqkv_t[:, bcol : bcol + 1], None,
                        mybir.AluOpType.add,
                    )
                qk_alt[0] += 1

            def emit_proj_q(b, tb, warmup=False):
                _proj_qk(b, tb, xq, "xq", wq_t, 0, q_b[b], warmup=warmup)

            def emit_proj_k(b, tb, warmup=False):
                _proj_qk(b, tb, xk, "xk", wk_t, 1, k_b[b], warmup=warmup)

            def emit_proj_v(b, tb):
                # v^T [128, TB] in psum -> +bias -> vT staging sbuf ->
                # 8 DMA-xbar transposes into the per-head token-major tiles.
                x_t = xinp.tile([128, KT, TB], BF16, tag="xv", name="xv_t", bufs=2)
                nc.gpsimd.dma_start(out=x_t[:], in_=xv[b * TPB + tb, :, :, :])
                ps_v = prjp.tile([128, TB], FP32, tag="prj", name="ps_v")
                for kt in range(KT):
                    nc.tensor.matmul(
                        ps_v[:, :], wv_t[:, kt, :], x_t[:, kt, :],
                        start=(kt == 0), stop=(kt == KT - 1),
                    )
                vT = vstgp.tile([128, TB], BF16, tag="vT", name="vT")
                nc.vector.tensor_scalar(
                    vT[:, :], ps_v[:, :], bqkv_t[:, 2:3], None,
                    mybir.AluOpType.add,
                )
                for sub in range(TB // 128):
                    jt = tb * (TB // 128) + sub
                    eng = nc.sync if tr_alt[0] % 2 == 0 else nc.scalar
                    tr_alt[0] += 1
                    eng.dma_start_transpose(
                        out=v_all[b][jt][:, :],
                        in_=vT[:, sub * 128 : (sub + 1) * 128],
                    )

            def emit_attention(b, inject=None):
                for ic in range(NIC):
                    at0 = attnp.tile([128, JT, IC], BF16, tag="at0", name="at0")
                    at1 = attnp.tile([128, JT, IC], BF16, tag="at1", name="at1")
                    pv_ps = pvp.tile([128, 2, 512], FP32, tag="pv", name="pv")
                    # one-bank den tile: 4 accumulation chains at partition
                    # offsets 0/32/64/96 = (h0,ih0),(h1,ih0),(h0,ih1),(h1,ih1)
                    dn_ps = dnp.tile([97, 512], FP32, tag="dn", name="dn")
                    pending = []

                    def make_pv(jt, at0=at0, at1=at1, pv_ps=pv_ps, dn_ps=dn_ps, b=b):
                        def go():
                            # pv pairs (col groups 0-1 vs 2-3) adjacent, then
                            # all four den chains (col groups 0/1/2/3) adjacent
                            for ih in range(2):
                                rh0 = at0[:, jt, ih * 512 : (ih + 1) * 512]
                                rh1 = at1[:, jt, ih * 512 : (ih + 1) * 512]
                                nc.tensor.matmul(
                                    pv_ps[0:DH, ih, :], v_all[b][jt][:, 0:DH], rh0,
                                    start=(jt == 0), stop=(jt == JT - 1),
                                    tile_position=(0, 0),
                                )
                                nc.tensor.matmul(
                                    pv_ps[DH:128, ih, :], v_all[b][jt][:, DH:128], rh1,
                                    start=(jt == 0), stop=(jt == JT - 1),
                                    tile_position=(0, 64),
                                )
                            for ih in range(2):
                                rh0 = at0[:, jt, ih * 512 : (ih + 1) * 512]
                                rh1 = at1[:, jt, ih * 512 : (ih + 1) * 512]
                                p0 = 64 * ih
                                nc.tensor.matmul(
                                    dn_ps[p0 : p0 + 1, :], ones_t[:, 0:1], rh0,
                                    start=(jt == 0), stop=(jt == JT - 1),
                                    tile_position=(0, p0),
                                )
                                nc.tensor.matmul(
                                    dn_ps[p0 + 32 : p0 + 33, :], ones_t[:, 0:1], rh1,
                                    start=(jt == 0), stop=(jt == JT - 1),
                                    tile_position=(0, p0 + 32),
                                )
                        return go

                    for jt in range(JT):
                        if inject is not None:
                            inject(ic, jt)
                        s0 = sc0p.tile([128, IC], FP32, tag="s0", name="s0")
                        s1 = sc1p.tile([128, IC], FP32, tag="s1", name="s1")
                        # adjacent emission -> the two heads' K=64 matmuls
                        # run concurrently in row groups 0-63 / 64-127
                        for n in range(IC // 512):
                            i0 = ic * IC + n * 512
                            nc.tensor.matmul(
                                s0[:, n * 512 : (n + 1) * 512],
                                k_b[b][0:DH, jt * 128 : (jt + 1) * 128],
                                q_b[b][0:DH, i0 : i0 + 512],
                                start=True, stop=True,
                            )
                            nc.tensor.matmul(
                                s1[:, n * 512 : (n + 1) * 512],
                                k_b[b][DH:128, jt * 128 : (jt + 1) * 128],
                                q_b[b][DH:128, i0 : i0 + 512],
                                start=True, stop=True,
                            )
                        # h0: table exp on ScalarE; h1: Schraudolph on VectorE
                        nc.scalar.activation(
                            out=at0[:, jt, :], in_=s0[:, :],
                            func=mybir.ActivationFunctionType.Exp,
                            scale=SCALE,
                        )
                        nc.vector.tensor_scalar(
                            at1[:, jt, :].bitcast(I16), s1[:, :],
                            SCH_A, SCH_B,
                            mybir.AluOpType.mult, mybir.AluOpType.add,
                        )
                        # pv/den of jt-1 AFTER this jt's scores+exp dispatch:
                        # their deps are ready, and the exp round-trip (which
                        # sets the period) isn't delayed behind them
                        if pending:
                            pending.pop()()
                        pending.append(make_pv(jt))
                    while pending:
                        pending.pop()()
                    # evacuate numerators (ScalarE) + denominators (VectorE)
                    pvsb = outp.tile([128, 2, 512], FP32, tag="pvsb", name="pvsb")
                    nc.scalar.copy(pvsb[:, :, :], pv_ps[:, :, :])
                    dnsb = outp.tile([97, 512], FP32, tag="dnsb", name="dnsb")
                    nc.vector.tensor_copy(dnsb[:, :], dn_ps[:, :])
                    c0 = b * S + ic * IC
                    nc.sync.dma_start(
                        out=out[0:128, c0 : c0 + IC],
                        in_=pvsb[:, :, :].rearrange("p a b -> p (a b)"),
                    )
                    for ih in range(2):
                        p0 = 64 * ih
                        nc.sync.dma_start(
                            out=out[128:129, c0 + ih * 512 : c0 + (ih + 1) * 512],
                            in_=dnsb[p0 : p0 + 1, :],
                        )
                        nc.sync.dma_start(
                            out=out[129:130, c0 + ih * 512 : c0 + (ih + 1) * 512],
                            in_=dnsb[p0 + 32 : p0 + 33, :],
                        )

            # ---- schedule ----
            # Pre-attention: k/q for the first i-chunk and j-tiles 0:8,
            # first v unit; the rest injected with >=4 points of slack
            # before their first consumer (the PE pulls LDWEIGHTS ahead
            # of in-flight matmuls, so tight evac->read gaps are unsafe).
            emit_proj_k(0, 0, warmup=True)
            emit_proj_q(0, 0)
            emit_proj_k(0, 1)
            emit_proj_q(0, 1)
            emit_proj_v(0, 0)

            units0 = [
                (0, lambda: emit_proj_v(0, 1)),
                (2, lambda: emit_proj_k(0, 2)),
                (4, lambda: emit_proj_v(0, 2)),
                (6, lambda: emit_proj_k(0, 3)),
                (8, lambda: emit_proj_v(0, 3)),
                (10, lambda: emit_proj_q(0, 2)),
                (12, lambda: emit_proj_q(0, 3)),
            ]
            units1 = []
            if B > 1:
                pts = [12, 14, 16, 18, 20, 22, 24, 26, 28, 30]
                seq = [
                    lambda: emit_proj_k(1, 0), lambda: emit_proj_k(1, 1),
                    lambda: emit_proj_k(1, 2), lambda: emit_proj_k(1, 3),
                    lambda: emit_proj_v(1, 0), lambda: emit_proj_v(1, 1),
                    lambda: emit_proj_q(1, 0), lambda: emit_proj_q(1, 1),
                    lambda: emit_proj_v(1, 2), lambda: emit_proj_v(1, 3),
                ]
                units0 += list(zip(pts, seq))
                units1 = [
                    (2, lambda: emit_proj_q(1, 2)),
                    (6, lambda: emit_proj_q(1, 3)),
                ]

            def make_inject(units):
                units = sorted(units, key=lambda u: u[0])
                ui = [0]

                def inject(ic, jt):
                    point = ic * JT + jt
                    while ui[0] < len(units) and units[ui[0]][0] <= point:
                        units[ui[0]][1]()
                        ui[0] += 1

                def flush():
                    while ui[0] < len(units):
                        units[ui[0]][1]()
                        ui[0] += 1

                return inject, flush

            inj0, flush0 = make_inject(units0)
            emit_attention(0, inject=inj0)
            flush0()
            if B > 1:
                inj1, flush1 = make_inject(units1)
                emit_attention(1, inject=inj1)
                flush1()

    nc.finalize()
    return nc


_PROGRAM_CACHE = {}


def _get_program(S, B):
    key = (S, B)
    if key not in _PROGRAM_CACHE:
        _PROGRAM_CACHE[key] = build_program(S, B)
    return _PROGRAM_CACHE[key]


def make_in_maps(query, key, value, Wq, bq, Wk, bk, Wv, bv):
    S, B, D_ = query.shape
    assert D_ == D
    T = S * B
    TB = 512
    NTILE = T // TB

    def xt(a):
        aT = np.asarray(a, np.float32).transpose(2, 1, 0).reshape(D_, T)
        a4 = aT.reshape(KT, 128, NTILE, TB).transpose(2, 1, 0, 3)
        return np.ascontiguousarray(a4).astype(NP_BF16)

    xqh, xkh, xvh = xt(query), xt(key), xt(value)

    def wt(W, rows):
        # [D, DC] col-slice -> [128, KT, DC] (partition-major contraction)
        wT = np.asarray(W)[rows, :].T.reshape(KT, 128, DC).transpose(1, 0, 2)
        return np.ascontiguousarray(wT).astype(NP_BF16)

    in_maps = []
    for c in range(NCORES):
        rows = slice(c * DC, (c + 1) * DC)
        in_maps.append(
            {
                "xq": xqh, "xk": xkh, "xv": xvh,
                "wq": wt(Wq, rows),
                "wk": wt(Wk, rows),
                "wv": wt(Wv, rows),
                "bqkv": np.ascontiguousarray(
                    np.stack(
                        [np.asarray(bq)[rows], np.asarray(bk)[rows], np.asarray(bv)[rows]],
                        axis=1,
                    )
                ).astype(np.float32),
            }
        )
    return in_maps


def gather_output(results, S, B):
    full = np.empty((S, B, D), np.float32)
    for c in range(NCORES):
        o = np.asarray(results[c]["out"], np.float32)  # [130, B*S]
        num = o[0:128]                                 # [128, T]
        den = np.empty((128, S * B), np.float32)
        den[0:DH] = o[128:129]
        den[DH:128] = o[129:130]
        res = (num / den).reshape(128, B, S).transpose(2, 1, 0)
        full[:, :, c * DC : (c + 1) * DC] = res
    return full


def kernel(query, key, value, Wq, bq, Wk, bk, Wv, bv):
    from concourse.bass_utils import run_bass_kernel_spmd

    S, B, _ = query.shape
    nc = _get_program(S, B)
    in_maps = make_in_maps(query, key, value, Wq, bq, Wk, bk, Wv, bv)
    res = run_bass_kernel_spmd(nc, in_maps, list(range(NCORES)))
    return gather_output(res.results, S, B)
